# revision 4
# baseline (speedup 1.0000x reference)
"""Trainium2 Bass kernel for nn_BinarizedCIFARNetwork.

Strategy:
  - Data-parallel conv trunk: batch 128 sharded 8 ways (16 samples/core).
    Activations binarized {0,1} and weights {-1,+1} are exact in bf16, so all
    binary conv layers run as bf16 matmuls (3x3 conv = 9 shift-accumulated
    matmuls into PSUM). conv0 (continuous input, 3 channels) runs as one
    im2col fp32 matmul (K=27).
  - BN+ReLU+sign(x) collapses to (x > mean) when beta==0 and gamma>0 (both
    guaranteed by setup_inputs); bias terms cancel inside batch-norm means.
    Batch stats need one tiny AllReduce per layer (sums per channel).
  - FC layers sharded by output features (weights pre-sliced per core on
    host); activations all-gathered (binarized, small). fc8 + log_softmax in
    fp32, computed redundantly on every core.
Host-side prep only reshapes/pads/shards the raw input arrays (no math).
"""

import numpy as np

N_CORES = 8
S = 16  # samples per core
EPS = 1e-5

_CACHE = {}


# ---------------------------------------------------------------------------
# Tile framework compatibility patches for this container's walrus build:
# it accepts only ONE sem-wait command per instruction.
# ---------------------------------------------------------------------------
def _patch_tile():
    if _CACHE.get("patched"):
        return
    import concourse.tile as tile_mod
    import concourse.mybir as mybir
    from concourse.tile import ScopedClock

    MAX_WAITS = 1

    def _drain_and_barrier(self, tick_clock, wait_clock):
        drain_inst = self.nc.sync.drain(fusable=False)
        wait_clock.add_sem_waits(
            drain_inst.ins, ScopedClock({None: tick_clock.global_clock})
        )
        si = drain_inst.ins.sync_info
        if si is not None and si.on_wait is not None and len(si.on_wait) > MAX_WAITS:
            waits = list(si.on_wait)
            drain_inst.ins.sync_info = mybir.SyncInfo(
                on_wait=waits[:MAX_WAITS], on_update=list(si.on_update or [])
            )
            for i in range(MAX_WAITS, len(waits), MAX_WAITS):
                d2 = self.nc.sync.drain(fusable=False)
                d2.ins.sync_info = mybir.SyncInfo(
                    on_wait=waits[i : i + MAX_WAITS], on_update=[]
                )
        self.nc.all_engine_barrier()
        assert self.sems is not None
        popped = self.nc._tile_sem_poison_stack.pop()
        assert popped is self._sem_poison
        self.nc.clear_and_free_semaphores(list(self.sems.allocated().values()))
        self.nc.all_engine_barrier()

    tile_mod.TileContext._drain_and_barrier = _drain_and_barrier

    _orig_lower = tile_mod.TileContext._lower_ordered_insts

    def _split_waits(self, ordered):
        for bb_name, insts in ordered.items():
            out = []
            for inst in insts:
                si = getattr(inst, "sync_info", None)
                try:
                    waits = list(si.on_wait) if (si is not None and si.on_wait) else []
                except Exception:
                    waits = []
                eng = getattr(inst, "engine", None)
                if len(waits) > MAX_WAITS and eng is not None:
                    extra, keep = waits[:-MAX_WAITS], waits[-MAX_WAITS:]
                    for i in range(0, len(extra), MAX_WAITS):
                        nop = mybir.InstNoOp(
                            name=self.nc.get_next_instruction_name(),
                            sync_info=mybir.SyncInfo(
                                on_wait=extra[i : i + MAX_WAITS], on_update=[]
                            ),
                            bass_nofuse=True,
                            engine=eng,
                        )
                        out.append(nop)
                    inst.sync_info = mybir.SyncInfo(
                        on_wait=keep, on_update=list(si.on_update or [])
                    )
                out.append(inst)
            ordered[bb_name] = out

    def _lower_ordered_insts(self, ordered):
        _split_waits(self, ordered)
        return _orig_lower(self, ordered)

    tile_mod.TileContext._lower_ordered_insts = _lower_ordered_insts
    _CACHE["patched"] = True


# ---------------------------------------------------------------------------
# Device program
# ---------------------------------------------------------------------------
def _build_program():
    if "nc" in _CACHE:
        return _CACHE["nc"]
    _patch_tile()
    import concourse.bass as bass
    import concourse.mybir as mybir
    import concourse.tile as tile

    F32 = mybir.dt.float32
    BF16 = mybir.dt.bfloat16
    ALU = mybir.AluOpType
    AX = mybir.AxisListType
    ACTF = mybir.ActivationFunctionType
    RG = [list(range(N_CORES))]

    nc = bass.Bass("TRN2", target_bir_lowering=False, debug=False,
                   num_devices=N_CORES)

    # ---- I/O -----------------------------------------------------------
    xp = nc.dram_tensor("xp", [64 + S * 3 * 1156 + 64], F32, kind="ExternalInput")
    w0t = nc.dram_tensor("w0t", [32, 128], F32, kind="ExternalInput")
    wts = {}
    conv_cfg = {
        1: dict(I=128, O=128, H=32, pool=True),
        2: dict(I=128, O=256, H=16, pool=False),
        3: dict(I=256, O=256, H=16, pool=True),
        4: dict(I=256, O=512, H=8, pool=False),
        5: dict(I=512, O=512, H=8, pool=True),
    }
    for l, cfg in conv_cfg.items():
        wts[l] = nc.dram_tensor(f"w{l}t", [3, 3, cfg["I"], cfg["O"]], F32,
                                kind="ExternalInput")
    w6tc = nc.dram_tensor("w6tc", [8192, 128], F32, kind="ExternalInput")
    w7tc = nc.dram_tensor("w7tc", [1024, 128], F32, kind="ExternalInput")
    w8t = nc.dram_tensor("w8t", [1024, 10], F32, kind="ExternalInput")
    b8d = nc.dram_tensor("b8", [10], F32, kind="ExternalInput")
    g7c = nc.dram_tensor("g7c", [128], F32, kind="ExternalInput")
    be7c = nc.dram_tensor("be7c", [128], F32, kind="ExternalInput")
    out_d = nc.dram_tensor("out", [128, 10], F32, kind="ExternalOutput")

    with tile.TileContext(nc, num_cores=N_CORES) as tc:
        # persistent pools
        ps = tc.alloc_tile_pool(name="ps", bufs=4, space="PSUM")
        dram = tc.alloc_tile_pool(name="dram", bufs=1, space="DRAM")
        small = tc.alloc_tile_pool(name="small", bufs=1)

        def ar_threshold(loc, MC, scale, lname):
            """AllReduce local per-channel sums; return thresholds [128, MC]."""
            cin = dram.tile([128, MC], F32, name=f"ar_in_{lname}")
            cout = dram.tile([128, MC], F32, name=f"ar_out_{lname}")
            nc.sync.dma_start(cin[:], loc[:])
            nc.gpsimd.collective_compute(
                "AllReduce", ALU.add, replica_groups=RG,
                ins=[cin.opt()], outs=[cout.opt()],
            )
            art = small.tile([128, MC], F32, name=f"art_{lname}")
            nc.sync.dma_start(art[:], cout[:])
            thr = small.tile([128, MC], F32, name=f"thr_{lname}")
            nc.vector.tensor_scalar(thr[:], art[:], scale, None, ALU.mult)
            return thr

        def binarize_into(in_t, stage_t, thr_col, Hp, H):
            """Zero borders of padded input tile, write (stage > thr) interior."""
            nc.gpsimd.memset(in_t[:, :, 0, :], 0.0)
            nc.gpsimd.memset(in_t[:, :, Hp - 1, :], 0.0)
            nc.gpsimd.memset(in_t[:, :, :, 0], 0.0)
            nc.gpsimd.memset(in_t[:, :, :, Hp - 1], 0.0)
            nc.vector.tensor_scalar(
                in_t[:, :, 1 : H + 1, 1 : H + 1], stage_t[:], thr_col, None,
                ALU.is_gt,
            )

        # ================= conv0: im2col fp32, K=27(->32) ================
        # Pool nesting is strictly LIFO: pa_{l+1} opens before pl_l so each
        # layer's scratch pool can be released immediately after use.
        pa1 = tc.alloc_tile_pool(name="pa1", bufs=1)
        in1 = pa1.tile([128, S, 34, 34], BF16, name="in1")

        pl0 = tc.alloc_tile_pool(name="pl0", bufs=1)
        w0st = pl0.tile([32, 128], F32, name="w0st")
        nc.sync.dma_start(w0st[:], w0t.ap())
        w0s = pl0.tile([32, 128], F32, name="w0s")
        nc.scalar.activation(w0s[:], w0st[:], ACTF.Sign)
        stage0 = pl0.tile([128, S, 32, 32], F32, name="stage0")
        sums0 = small.tile([128, 32], F32, name="sums0")
        nc.vector.memset(sums0[:], 0.0)

        for chunk in range(2):  # 8 samples at a time (SBUF)
            s0 = chunk * 8
            rhs = pl0.tile([32, 8, 34, 34], F32, name="rhs27", tag="rhs27")
            if chunk == 0:
                # zero whole tile once; rows 27-31 stay zero for both chunks
                # (same tag+bufs=1 slot), rows 0-26 are DMA-overwritten
                nc.vector.memset(rhs[:], 0.0)
            for dd in range(9):
                dy, dx = dd // 3 - 1, dd % 3 - 1
                off = 64 + dy * 34 + dx + s0 * 3468
                src = xp.ap()[off : off + 8 * 3468].rearrange(
                    "(s c e) -> c s e", s=8, c=3
                )
                nc.sync.dma_start(rhs[3 * dd : 3 * dd + 3, :, :, :], src)
            for t in range(16):
                s, h = t // 2, t % 2
                psum = ps.tile([128, 16, 32], F32, name="ps0", tag="ps")
                nc.tensor.matmul(
                    psum[:], w0s[:],
                    rhs[:, s, 1 + 16 * h : 17 + 16 * h, 1:33],
                    start=True, stop=True,
                )
                nc.vector.tensor_scalar(
                    stage0[:, s0 + s, 16 * h : 16 * h + 16, :], psum[:],
                    0.0, 0.0, ALU.add, ALU.add,
                    accum_out=sums0[:, chunk * 16 + t : chunk * 16 + t + 1],
                )
        loc0 = small.tile([128, 1], F32, name="loc0")
        nc.vector.reduce_sum(loc0[:], sums0[:], axis=AX.X)
        thr0 = ar_threshold(loc0, 1, 1.0 / (128 * 1024), "l0")
        binarize_into(in1, stage0, thr0[:, 0:1], 34, 32)
        pl0.release()

        # ================= conv layers 1..5 (bf16 binary) ================
        in_tiles = {1: [in1]}
        act_pools = [pa1]
        h5b = None

        for l, cfg in conv_cfg.items():
            I, O, H, pool = cfg["I"], cfg["O"], cfg["H"], cfg["pool"]
            KC, MC = I // 128, O // 128
            Hp = H + 2
            Ho = H // 2 if pool else H  # stats/bin act size

            # next-layer activation pool first (outlives this layer's scratch)
            if l < 5:
                Hn = conv_cfg[l + 1]["H"]
                pa_next = tc.alloc_tile_pool(name=f"pa{l + 1}", bufs=1)
                nxt = [pa_next.tile([128, S, Hn + 2, Hn + 2], BF16,
                                    name=f"in{l + 1}_{mc}") for mc in range(MC)]
            else:
                pa_next = tc.alloc_tile_pool(name="pa_h5", bufs=1)
                h5b = [pa_next.tile([128, S, 16], BF16, name=f"h5b_{mc}")
                       for mc in range(4)]
            act_pools.append(pa_next)

            pl = tc.alloc_tile_pool(name=f"pl{l}", bufs=1)

            # weights: [3,3,I,O] -> bf16 sign lhsT tiles [128, KC, 9, MC*128]
            wsb = pl.tile([128, KC, 9, MC * 128], BF16, name=f"w{l}sb")
            wt_ap = wts[l].ap().rearrange("ky kx i o -> i (ky kx) o")
            for kc in range(KC):
                wst = pl.tile([128, 9, MC * 128], F32, name=f"w{l}st{kc % 2}",
                              tag=f"wst{kc % 2}")
                nc.sync.dma_start(wst[:], wt_ap[kc * 128 : (kc + 1) * 128])
                nc.scalar.activation(
                    wsb[:, kc].rearrange("p a b -> p (a b)"),
                    wst[:].rearrange("p a b -> p (a b)"), ACTF.Sign,
                )

            stages = []
            sums_l = []
            if H == 32:
                ntiles = 32
            elif H == 16:
                ntiles = 8
            else:
                ntiles = 2
            for mc in range(MC):
                st = pl.tile([128, S, Ho, Ho], F32, name=f"stage{l}_{mc}")
                stages.append(st)
                sm = small.tile([128, ntiles], F32, name=f"sums{l}_{mc}")
                nc.vector.memset(sm[:], 0.0)
                sums_l.append(sm)

            ins = in_tiles[l]
            for mc in range(MC):
                for t in range(ntiles):
                    if H == 32:
                        psh = [128, 16, 32]
                    elif H == 16:
                        psh = [128, 2, 16, 16]
                    else:
                        psh = [128, 8, 8, 8]
                    psum = ps.tile(psh, F32, name=f"ps{l}", tag="ps")
                    n_acc = KC * 9
                    idx = 0
                    for kc in range(KC):
                        for dd in range(9):
                            dy, dx = dd // 3 - 1, dd % 3 - 1
                            if H == 32:
                                s, h = t // 2, t % 2
                                rhs = ins[kc][:, s,
                                              1 + dy + 16 * h : 17 + dy + 16 * h,
                                              1 + dx : 33 + dx]
                            elif H == 16:
                                s0 = 2 * t
                                rhs = ins[kc][:, s0 : s0 + 2,
                                              1 + dy : 17 + dy, 1 + dx : 17 + dx]
                            else:
                                s0 = 8 * t
                                rhs = ins[kc][:, s0 : s0 + 8,
                                              1 + dy : 9 + dy, 1 + dx : 9 + dx]
                            nc.tensor.matmul(
                                psum[:],
                                wsb[:, kc, dd, mc * 128 : (mc + 1) * 128],
                                rhs,
                                start=(idx == 0), stop=(idx == n_acc - 1),
                            )
                            idx += 1
                    # evict (+ pool) + fused per-channel sum
                    acc = sums_l[mc][:, t : t + 1]
                    if not pool:
                        if H == 16:
                            dst = stages[mc][:, 2 * t : 2 * t + 2, :, :]
                        else:
                            dst = stages[mc][:, 8 * t : 8 * t + 8, :, :]
                        nc.vector.tensor_scalar(
                            dst, psum[:], 0.0, 0.0, ALU.add, ALU.add,
                            accum_out=acc,
                        )
                    elif H == 32:
                        s, h = t // 2, t % 2
                        pv = psum[:].rearrange("p y (x two) -> p y x two", two=2)
                        tmpx = pl.tile([128, 16, 16], F32, name=f"tmpx{l}",
                                       tag="tmpx")
                        nc.vector.reduce_max(tmpx[:], pv, axis=AX.X)
                        tv = tmpx[:].rearrange("p (yp two) x -> p yp two x",
                                               two=2)
                        nc.vector.scalar_tensor_tensor(
                            stages[mc][:, s, 8 * h : 8 * h + 8, :],
                            tv[:, :, 0, :], 0.0, tv[:, :, 1, :],
                            ALU.add, ALU.max, accum_out=acc,
                        )
                    elif H == 16:
                        s0 = 2 * t
                        pv = psum[:].rearrange(
                            "p s y (xp two) -> p (s y) xp two", two=2)
                        tmpx = pl.tile([128, 32, 8], F32, name=f"tmpx{l}",
                                       tag="tmpx")
                        nc.vector.reduce_max(tmpx[:], pv, axis=AX.X)
                        tv = tmpx[:].rearrange(
                            "p (s yp two) xp -> p s yp two xp", s=2, two=2)
                        nc.vector.scalar_tensor_tensor(
                            stages[mc][:, s0 : s0 + 2, :, :],
                            tv[:, :, :, 0, :], 0.0, tv[:, :, :, 1, :],
                            ALU.add, ALU.max, accum_out=acc,
                        )
                    else:
                        s0 = 8 * t
                        pv = psum[:].rearrange(
                            "p s y (xp two) -> p (s y) xp two", two=2)
                        tmpx = pl.tile([128, 64, 4], F32, name=f"tmpx{l}",
                                       tag="tmpx")
                        nc.vector.reduce_max(tmpx[:], pv, axis=AX.X)
                        tv = tmpx[:].rearrange(
                            "p (s yp two) xp -> p s yp two xp", s=8, two=2)
                        nc.vector.scalar_tensor_tensor(
                            stages[mc][:, s0 : s0 + 8, :, :],
                            tv[:, :, :, 0, :], 0.0, tv[:, :, :, 1, :],
                            ALU.add, ALU.max, accum_out=acc,
                        )

            # stats -> AllReduce -> thresholds -> binarize into next input
            loc = small.tile([128, MC], F32, name=f"loc{l}")
            for mc in range(MC):
                nc.vector.reduce_sum(loc[:, mc : mc + 1], sums_l[mc][:], axis=AX.X)
            thr = ar_threshold(loc, MC, 1.0 / (128 * Ho * Ho), f"l{l}")

            if l < 5:
                Hn = conv_cfg[l + 1]["H"]
                for mc in range(MC):
                    binarize_into(nxt[mc], stages[mc], thr[:, mc : mc + 1],
                                  Hn + 2, Hn)
                in_tiles[l + 1] = nxt
            else:
                for mc in range(4):
                    nc.vector.tensor_scalar(
                        h5b[mc][:],
                        stages[mc][:].rearrange("p s a b -> p s (a b)"),
                        thr[:, mc : mc + 1], None, ALU.is_gt,
                    )
            pl.release()

        # ================= FC section ===================================
        fcp = tc.alloc_tile_pool(name="fcp", bufs=1)

        # all-gather h5b (binarized conv output, bf16)
        ag5_in = dram.tile([4, 128, S * 16], BF16, name="ag5_in")
        ag5_out = dram.tile([N_CORES, 4, 128, S * 16], BF16, name="ag5_out")
        for mc in range(4):
            nc.sync.dma_start(ag5_in[mc], h5b[mc][:])
        nc.gpsimd.collective_compute(
            "AllGather", ALU.bypass, replica_groups=RG,
            ins=[ag5_in.opt()], outs=[ag5_out.opt()],
        )
        h5g = []
        for mc in range(4):
            t = fcp.tile([128, N_CORES, S, 16], BF16, name=f"h5g_{mc}")
            src = ag5_out[:, mc, :, :].rearrange("r c e -> c r e")
            nc.sync.dma_start(t[:].rearrange("p r s e -> p r (s e)"), src)
            h5g.append(t)

        # fc6 weights: w6tc [8192, 128] -> sign bf16 [128, 4, 16, 128]
        w6sb = fcp.tile([128, 4, 16, 128], BF16, name="w6sb")
        w6src = w6tc.ap().rearrange("(mc c sp) o -> c mc sp o", mc=4, c=128)
        for mc in range(4):
            w6st = fcp.tile([128, 16, 128], F32, name=f"w6st{mc % 2}",
                            tag=f"w6st{mc % 2}")
            nc.sync.dma_start(w6st[:], w6src[:, mc])
            nc.scalar.activation(
                w6sb[:, mc].rearrange("p a b -> p (a b)"),
                w6st[:].rearrange("p a b -> p (a b)"), ACTF.Sign,
            )

        psum6 = ps.tile([128, N_CORES, S], F32, name="ps6", tag="ps")
        idx = 0
        for mc in range(4):
            for sp in range(16):
                nc.tensor.matmul(
                    psum6[:], w6sb[:, mc, sp, :], h5g[mc][:, :, :, sp],
                    start=(idx == 0), stop=(idx == 63),
                )
                idx += 1
        z6 = fcp.tile([128, 128], F32, name="z6")
        s6 = small.tile([128, 1], F32, name="s6")
        nc.vector.memset(s6[:], 0.0)
        nc.vector.tensor_scalar(
            z6[:], psum6[:].rearrange("p a b -> p (a b)"),
            0.0, 0.0, ALU.add, ALU.add, accum_out=s6[:],
        )
        m6 = small.tile([128, 1], F32, name="m6")
        nc.vector.tensor_scalar(m6[:], s6[:], 1.0 / 128, None, ALU.mult)
        h6b = fcp.tile([128, 128], BF16, name="h6b")
        nc.vector.tensor_scalar(h6b[:], z6[:], m6[:], None, ALU.is_gt)

        # all-gather h6b
        ag6_in = dram.tile([128, 128], BF16, name="ag6_in")
        ag6_out = dram.tile([N_CORES, 128, 128], BF16, name="ag6_out")
        nc.sync.dma_start(ag6_in[:], h6b[:])
        nc.gpsimd.collective_compute(
            "AllGather", ALU.bypass, replica_groups=RG,
            ins=[ag6_in.opt()], outs=[ag6_out.opt()],
        )
        h6g = fcp.tile([128, N_CORES, 128], BF16, name="h6g")
        nc.sync.dma_start(h6g[:], ag6_out[:, :, :].rearrange("r p b -> p r b"))

        # fc7
        w7st = fcp.tile([128, N_CORES, 128], F32, name="w7st")
        nc.sync.dma_start(
            w7st[:], w7tc.ap().rearrange("(r c) o -> c r o", c=128))
        w7sb = fcp.tile([128, N_CORES, 128], BF16, name="w7sb")
        nc.scalar.activation(
            w7sb[:].rearrange("p a b -> p (a b)"),
            w7st[:].rearrange("p a b -> p (a b)"), ACTF.Sign,
        )
        psum7 = ps.tile([128, 128], F32, name="ps7", tag="ps")
        for r in range(N_CORES):
            nc.tensor.matmul(psum7[:], w7sb[:, r, :], h6g[:, r, :],
                             start=(r == 0), stop=(r == N_CORES - 1))
        z7 = fcp.tile([128, 128], F32, name="z7")
        s7 = small.tile([128, 1], F32, name="s7")
        nc.vector.memset(s7[:], 0.0)
        nc.vector.tensor_scalar(z7[:], psum7[:], 0.0, 0.0, ALU.add, ALU.add,
                                accum_out=s7[:])
        m7 = small.tile([128, 1], F32, name="m7")
        nc.vector.tensor_scalar(m7[:], s7[:], 1.0 / 128, None, ALU.mult)
        sq7 = fcp.tile([128, 128], F32, name="sq7")
        ss7 = small.tile([128, 1], F32, name="ss7")
        nc.vector.memset(ss7[:], 0.0)
        nc.scalar.activation(sq7[:], z7[:], ACTF.Square, accum_out=ss7[:])
        # rstd = 1/sqrt(ss7/128 - m7^2 + eps); h7 = relu((z7-m7)*g*rstd + be)
        v7 = small.tile([128, 1], F32, name="v7")
        nc.vector.tensor_scalar(v7[:], ss7[:], 1.0 / 128, None, ALU.mult)
        m7sq = small.tile([128, 1], F32, name="m7sq")
        nc.vector.tensor_tensor(m7sq[:], m7[:], m7[:], ALU.mult)
        nc.vector.tensor_tensor(v7[:], v7[:], m7sq[:], ALU.subtract)
        nc.vector.tensor_scalar(v7[:], v7[:], EPS, None, ALU.add)
        sd7 = small.tile([128, 1], F32, name="sd7")
        nc.scalar.activation(sd7[:], v7[:], ACTF.Sqrt)
        rstd7 = small.tile([128, 1], F32, name="rstd7")
        nc.vector.reciprocal(rstd7[:], sd7[:])
        g7s = small.tile([128, 1], F32, name="g7s")
        nc.sync.dma_start(g7s[:], g7c.ap().rearrange("(p one) -> p one", one=1))
        be7s = small.tile([128, 1], F32, name="be7s")
        nc.sync.dma_start(be7s[:], be7c.ap().rearrange("(p one) -> p one", one=1))
        a7 = small.tile([128, 1], F32, name="a7")
        nc.vector.tensor_tensor(a7[:], g7s[:], rstd7[:], ALU.mult)
        nm7 = small.tile([128, 1], F32, name="nm7")
        nc.vector.tensor_tensor(nm7[:], m7[:], a7[:], ALU.mult)
        b7t = small.tile([128, 1], F32, name="b7t")
        nc.vector.tensor_tensor(b7t[:], be7s[:], nm7[:], ALU.subtract)
        h7 = fcp.tile([128, 128], F32, name="h7")
        nc.scalar.activation(h7[:], z7[:], ACTF.Relu, bias=b7t[:], scale=a7[:])

        # all-gather h7 (fp32)
        ag7_in = dram.tile([128, 128], F32, name="ag7_in")
        ag7_out = dram.tile([N_CORES, 128, 128], F32, name="ag7_out")
        nc.sync.dma_start(ag7_in[:], h7[:])
        nc.gpsimd.collective_compute(
            "AllGather", ALU.bypass, replica_groups=RG,
            ins=[ag7_in.opt()], outs=[ag7_out.opt()],
        )
        h7g = fcp.tile([128, N_CORES, 128], F32, name="h7g")
        nc.sync.dma_start(h7g[:], ag7_out[:, :, :].rearrange("r p b -> p r b"))

        # fc8 (fp32) + bias via K=1 matmul + log_softmax
        w8sb = fcp.tile([128, N_CORES, 10], F32, name="w8sb")
        nc.sync.dma_start(w8sb[:], w8t.ap().rearrange("(r c) o -> c r o", c=128))
        ones1 = fcp.tile([1, 128], F32, name="ones1")
        nc.vector.memset(ones1[:], 1.0)
        b8sb = fcp.tile([1, 10], F32, name="b8sb")
        nc.sync.dma_start(b8sb[:], b8d.ap().rearrange("(one o) -> one o", one=1))
        psum8 = ps.tile([128, 10], F32, name="ps8", tag="ps")
        for r in range(N_CORES):
            nc.tensor.matmul(psum8[:], h7g[:, r, :], w8sb[:, r, :],
                             start=(r == 0), stop=False)
        nc.tensor.matmul(psum8[:], ones1[:], b8sb[:], start=False, stop=True)

        mx = small.tile([128, 1], F32, name="mx")
        nc.vector.reduce_max(mx[:], psum8[:], axis=AX.X)
        zc = fcp.tile([128, 10], F32, name="zc")
        nc.vector.tensor_scalar(zc[:], psum8[:], mx[:], None, ALU.subtract)
        e8 = fcp.tile([128, 10], F32, name="e8")
        se = small.tile([128, 1], F32, name="se")
        nc.vector.memset(se[:], 0.0)
        nc.scalar.activation(e8[:], zc[:], ACTF.Exp, accum_out=se[:])
        lse = small.tile([128, 1], F32, name="lse")
        nc.scalar.activation(lse[:], se[:], ACTF.Ln)
        outsb = fcp.tile([128, 10], F32, name="outsb")
        nc.vector.tensor_scalar(outsb[:], zc[:], lse[:], None, ALU.subtract)
        nc.sync.dma_start(out_d.ap(), outsb[:])

        fcp.release()
        for p in reversed(act_pools):
            p.release()
        small.release()
        dram.release()
        ps.release()

    _CACHE["nc"] = nc
    return nc


# ---------------------------------------------------------------------------
# Host wrapper
# ---------------------------------------------------------------------------
def kernel(trace=False, **inputs):
    from concourse import bass_utils

    x = np.asarray(inputs["x"], dtype=np.float32)
    for i in range(8):
        assert np.all(np.asarray(inputs[f"be{i}"]) == 0.0), "be!=0 unsupported"
        assert np.all(np.asarray(inputs[f"g{i}"]) > 0.0), "g<=0 unsupported"

    # pad x to 34x34 with zeros, flatten per-core with 64-elem guard bands
    xpad = np.zeros((128, 3, 34, 34), dtype=np.float32)
    xpad[:, :, 1:33, 1:33] = x
    guard = np.zeros(64, dtype=np.float32)

    w0 = np.asarray(inputs["w0"], dtype=np.float32)
    w0t = np.zeros((32, 128), dtype=np.float32)
    w0t[:27] = w0.transpose(2, 3, 1, 0).reshape(27, 128)

    wts = {}
    for l in range(1, 6):
        wts[l] = np.ascontiguousarray(
            np.asarray(inputs[f"w{l}"], dtype=np.float32).transpose(2, 3, 1, 0))

    w6T = np.ascontiguousarray(np.asarray(inputs["w6"], dtype=np.float32).T)
    w7T = np.ascontiguousarray(np.asarray(inputs["w7"], dtype=np.float32).T)
    w8T = np.ascontiguousarray(np.asarray(inputs["w8"], dtype=np.float32).T)
    b8 = np.ascontiguousarray(np.asarray(inputs["b8"], dtype=np.float32))
    g7 = np.asarray(inputs["g7"], dtype=np.float32)
    be7 = np.asarray(inputs["be7"], dtype=np.float32)

    in_maps = []
    for c in range(N_CORES):
        xc = xpad[S * c : S * (c + 1)]
        m = {
            "xp": np.concatenate([guard, xc.ravel(), guard]),
            "w0t": w0t,
            "w6tc": np.ascontiguousarray(w6T[:, 128 * c : 128 * (c + 1)]),
            "w7tc": np.ascontiguousarray(w7T[:, 128 * c : 128 * (c + 1)]),
            "w8t": w8T,
            "b8": b8,
            "g7c": np.ascontiguousarray(g7[128 * c : 128 * (c + 1)]),
            "be7c": np.ascontiguousarray(be7[128 * c : 128 * (c + 1)]),
        }
        for l in range(1, 6):
            m[f"w{l}t"] = wts[l]
        in_maps.append(m)

    nc = _build_program()
    res = bass_utils.run_bass_kernel_spmd(
        nc, in_maps, core_ids=list(range(N_CORES)), trace=trace,
    )
    _CACHE["last_results"] = res
    return res.results[0]["out"]
